# revision 14
# baseline (speedup 1.0000x reference)
"""Trainium2 Bass kernel for nn_Baseline_GNN (gnn_message_passing).

Data-parallel over batch across 8 NeuronCores. Restructured pipeline
(v2): uses (M@h)@W1 == M@(h@W1) so the dense product consumes
feature-major h directly (no per-sample transposes anywhere), the mask
matmul emits z1 feature-major, and ELU3's output is already in the
layout the next layer needs.

  per layer l (3x):
    D1A:  hW = h @ W1 per 4-sample group (PE; stationary = bufC
          feature-major chunks, moving = W1 row-major) -> PSUM ->
          hW group tile fp16 (ACT/DVE alternating copies).
    MASK: z1.T = (maskT + eps*I)_s @ hW_s per sample (PE, 2 j-pieces)
          -> PSUM fp32 -> bufA fp16 copies.
    BN1:  subset stats (24/32 samples; sum via DVE ts-accum, sumsq via
          DVE stt-accum), AllReduce overlapped with tail samples.
    ELU1: shifted form u-t = max(min(exp(n),1)-1-t, s*z) -- the t
          offset is absorbed by the next BatchNorm (shift-invariant):
          1 ACT exp + 2 DVE passes.
    D2:   z2.T = W2.T @ u.T (PE), copies carry accum_out -> z2 sums.
    BN2/ELU2: as BN1/ELU1; ELU2's stt carries accum_out -> w sums.
    BN3:  sums from ELU2 accums, sumsq via DVE stt subset; AllReduce.
    ELU3: exact (3 DVE + 1 ACT) -> h' into bufC (feature-major).
  final: xm = row-sum over roi (DVE reduce; 1/200 folded into Wm1),
         AllGather xm, replicated tiny MLP with full-batch BN stats,
         y (256,2).

b1/b2/bm1 are mathematically dropped (train-mode BN subtracts the mean,
so per-feature constant biases cancel exactly).
"""
import numpy as np
import ml_dtypes

import concourse.bass as bass
import concourse.mybir as mybir
import concourse.tile as tile
import concourse.bacc as bacc
from concourse.bass_utils import run_bass_kernel_spmd

F32 = mybir.dt.float32
F16 = mybir.dt.float16
AF = mybir.ActivationFunctionType
A = mybir.AluOpType

B, ROI, T, L = 256, 200, 512, 3
NCORES = 8
S = B // NCORES            # samples per core (32)
RPC = S * ROI              # rows per core (6400)
FC = T // 128              # feature chunks (4)
GRP = 2                    # samples per hW group
NG = 16                    # groups
NBLK = (RPC + 511) // 512  # 13 dense row-blocks
NAB = (RPC + 1023) // 1024 # 7 elu blocks
BN_EPS = 1e-5

# rows used for BN stats (full local rows; graphs are correlated so
# sample-subset stats are too noisy -- measured 8e-2 on y with 24/32)
SUB1 = RPC
SUB2 = RPC
SUB3 = RPC


def build_nc():
    nc = bacc.Bacc("TRN2", target_bir_lowering=False, debug=False,
                   num_devices=NCORES)

    xrt = nc.dram_tensor("xrt", [128, FC, RPC], F16, kind="ExternalInput")
    mk = nc.dram_tensor("mk", [L, S, ROI, ROI], F16, kind="ExternalInput")
    w12 = nc.dram_tensor("w12", [L, 2, 128, FC, T], F16, kind="ExternalInput")
    bnp = nc.dram_tensor("bnp", [L, 6, 128, FC], F32, kind="ExternalInput")
    wm1 = nc.dram_tensor("wm1", [128, FC, 256], F16, kind="ExternalInput")
    wm2 = nc.dram_tensor("wm2", [128, 2, 2], F16, kind="ExternalInput")
    fbn = nc.dram_tensor("fbn", [128, 5], F32, kind="ExternalInput")
    y = nc.dram_tensor("y", [B, 2], F32, kind="ExternalOutput")
    import os
    DBG = os.environ.get("K_DBG", "") == "1"
    if DBG:
        dbg_h = nc.dram_tensor("dbg_h", [128, FC, RPC], F16,
                               kind="ExternalOutput")
        dbg_hw = nc.dram_tensor("dbg_hw", [128, GRP, 2, T], F16,
                                kind="ExternalOutput")
        dbg_z1 = nc.dram_tensor("dbg_z1", [128, FC, RPC], F16,
                                kind="ExternalOutput")
        dbg_u = nc.dram_tensor("dbg_u", [128, FC, RPC], F16,
                               kind="ExternalOutput")
        dbg_z2 = nc.dram_tensor("dbg_z2", [128, FC, RPC], F16,
                                kind="ExternalOutput")
        dbg_w = nc.dram_tensor("dbg_w", [128, FC, RPC], F16,
                               kind="ExternalOutput")
        dbg_h2 = nc.dram_tensor("dbg_h2", [128, FC, RPC], F16,
                                kind="ExternalOutput")

    with tile.TileContext(nc) as tc:
        with (
            tc.tile_pool(name="big", bufs=1) as big,
            tc.tile_pool(name="wts", bufs=1) as wts,
            tc.tile_pool(name="mskp", bufs=3) as mskp,
            tc.tile_pool(name="esc", bufs=3) as esc,
            tc.tile_pool(name="stt", bufs=4) as stt,
            tc.tile_pool(name="dram", bufs=1, space="DRAM") as dram,
            tc.tile_pool(name="aps", bufs=2, space="PSUM") as aps,
            tc.tile_pool(name="dps", bufs=4, space="PSUM") as dps,
        ):
            bufA = big.tile([128, FC, RPC], F16)   # z1T / z2T
            hw2 = big.tile([128, 3, GRP, 2, T], F16)  # hW slots (g%3)
            bufB = big.tile([128, FC, RPC], F16)   # uT / wT
            bufC = big.tile([128, FC, RPC], F16)   # h / h'
            junk = big.tile([128, 2560], F16)      # dead out for accum passes

            bnpt = big.tile([128, L, 6, FC], F32)
            nc.sync.dma_start(bnpt[:], bnp.ap().rearrange("l k p c -> p l k c"))
            fbnt = big.tile([128, 5], F32)
            nc.sync.dma_start(fbnt[:], fbn.ap())
            wm1t = big.tile([128, FC, 256], F16)
            nc.sync.dma_start(wm1t[:], wm1.ap())
            wm2t = big.tile([128, 2, 2], F16)
            nc.sync.dma_start(wm2t[:], wm2.ap())
            nc.sync.dma_start(bufC[:], xrt.ap())

            def ar_stats(sum_ap, sq_ap, nglob, l, gk, bek, tag):
                """AllReduce (sum, sumsq) -> (s, t, tm=-1-t), each [128,FC]."""
                pay = stt.tile([128, 2 * FC], F32, name=f"pay{tag}", tag="pay")
                nc.vector.tensor_copy(pay[:, :FC], sum_ap)
                nc.vector.tensor_copy(pay[:, FC:], sq_ap)
                bin_ = dram.tile([128, 2 * FC], F32, name=f"bin{tag}")
                bout = dram.tile([128, 2 * FC], F32, name=f"bout{tag}",
                                 addr_space="Shared")
                nc.sync.dma_start(bin_[:], pay[:])
                nc.gpsimd.collective_compute(
                    "AllReduce", A.add, ins=[bin_[:].opt()], outs=[bout[:].opt()],
                    replica_groups=[list(range(NCORES))])
                gp = stt.tile([128, 2 * FC], F32, name=f"gp{tag}", tag="gp")
                nc.sync.dma_start(gp[:], bout[:])
                mg = stt.tile([128, FC], F32, name=f"mg{tag}", tag="mg")
                vg = stt.tile([128, FC], F32, name=f"vg{tag}", tag="vg")
                msq = stt.tile([128, FC], F32, name=f"msq{tag}", tag="msq")
                nc.vector.tensor_scalar(mg[:], gp[:, :FC], 1.0 / nglob, 0.0,
                                        A.mult, A.add)
                nc.vector.tensor_scalar(vg[:], gp[:, FC:], 1.0 / nglob, 0.0,
                                        A.mult, A.add)
                nc.vector.tensor_tensor(msq[:], mg[:], mg[:], A.mult)
                nc.vector.tensor_tensor(vg[:], vg[:], msq[:], A.subtract)
                nc.vector.tensor_scalar(vg[:], vg[:], 1.0, BN_EPS, A.mult, A.add)
                nc.scalar.activation(vg[:], vg[:], AF.Ln, bias=0.0, scale=1.0)
                nc.scalar.activation(vg[:], vg[:], AF.Exp, bias=0.0, scale=-0.5)
                st_s = stt.tile([128, FC], F32, name=f"s{tag}", tag="s")
                st_t = stt.tile([128, FC], F32, name=f"t{tag}", tag="t")
                st_m = stt.tile([128, FC], F32, name=f"m{tag}", tag="m")
                nc.vector.tensor_tensor(st_s[:], vg[:], bnpt[:, l, gk], A.mult)
                nc.vector.tensor_tensor(msq[:], mg[:], st_s[:], A.mult)
                nc.vector.tensor_tensor(st_t[:], bnpt[:, l, bek], msq[:],
                                        A.subtract)
                nc.vector.tensor_scalar(st_m[:], st_t[:], -1.0, -1.0,
                                        A.mult, A.add)
                return st_s, st_t, st_m

            def elu_shift(zT, uT, st_s, st_t, st_m, tag, acc=None):
                """uT = ELU(s*z+t) - t = max(min(exp(n),1)-1-t, s*z)."""
                for rb in range(NAB):
                    off = rb * 1024
                    n = min(1024, RPC - off)
                    for fc in range(FC):
                        src = zT[:, fc, off:off + n]
                        e = esc.tile([128, 1024], F16, name=f"e{tag}_{rb}_{fc}",
                                     tag="eb")
                        nc.scalar.activation(e[:, :n], src, AF.Exp,
                                             bias=st_t[:, fc:fc + 1],
                                             scale=st_s[:, fc:fc + 1])
                        nc.vector.tensor_scalar(e[:, :n], e[:, :n], 1.0,
                                                st_m[:, fc:fc + 1],
                                                A.min, A.add)
                        nc.vector.scalar_tensor_tensor(
                            uT[:, fc, off:off + n], src, st_s[:, fc:fc + 1],
                            e[:, :n], A.mult, A.max,
                            accum_out=(acc[:, fc, rb:rb + 1]
                                       if acc is not None else None))

            def elu_exact(zT, uT, st_s, st_t, tag):
                """uT = ELU(s*z+t) = max(min(exp(n),1)-1, n)."""
                for rb in range(NAB):
                    off = rb * 1024
                    n = min(1024, RPC - off)
                    for fc in range(FC):
                        src = zT[:, fc, off:off + n]
                        e = esc.tile([128, 1024], F16, name=f"e{tag}_{rb}_{fc}",
                                     tag="eb")
                        r = esc.tile([128, 1024], F16, name=f"r{tag}_{rb}_{fc}",
                                     tag="eb")
                        nc.scalar.activation(e[:, :n], src, AF.Exp,
                                             bias=st_t[:, fc:fc + 1],
                                             scale=st_s[:, fc:fc + 1])
                        nc.vector.tensor_scalar(r[:, :n], src,
                                                st_s[:, fc:fc + 1],
                                                st_t[:, fc:fc + 1],
                                                A.mult, A.add)
                        nc.vector.tensor_scalar(e[:, :n], e[:, :n], 1.0, -1.0,
                                                A.min, A.add)
                        nc.vector.tensor_tensor(uT[:, fc, off:off + n],
                                                r[:, :n], e[:, :n], A.max)

            def sumsq_subset(srcT, cols, tag):
                """DVE stt z*z with accum over [0:cols) per fc chunk."""
                nch = (cols + 2559) // 2560
                out = stt.tile([128, FC, nch], F32, name=f"ssq{tag}", tag="ssq")
                for fc in range(FC):
                    for k in range(nch):
                        c0 = k * 2560
                        n = min(2560, cols - c0)
                        nc.vector.scalar_tensor_tensor(
                            junk[:, :n], srcT[:, fc, c0:c0 + n], 1.0,
                            srcT[:, fc, c0:c0 + n], A.mult, A.mult,
                            accum_out=out[:, fc, k:k + 1])
                red = stt.tile([128, FC], F32, name=f"ssr{tag}", tag="ssr")
                nc.vector.tensor_reduce(red[:], out[:], mybir.AxisListType.X,
                                        A.add)
                return red

            def sum_subset(srcT, cols, tag):
                """DVE ts copy with accum over [0:cols) per fc chunk."""
                nch = (cols + 2559) // 2560
                out = stt.tile([128, FC, nch], F32, name=f"ss{tag}", tag="ss")
                for fc in range(FC):
                    for k in range(nch):
                        c0 = k * 2560
                        n = min(2560, cols - c0)
                        nc.vector.tensor_scalar(
                            junk[:, :n], srcT[:, fc, c0:c0 + n], 1.0, 0.0,
                            A.mult, A.add, accum_out=out[:, fc, k:k + 1])
                red = stt.tile([128, FC], F32, name=f"sr{tag}", tag="sr")
                nc.vector.tensor_reduce(red[:], out[:], mybir.AxisListType.X,
                                        A.add)
                return red

            # ================== main ==================
            for l in range(L):
                wt = wts.tile([128, 2, FC, T], F16, name=f"wt{l}", tag="wt")
                nc.sync.dma_start(wt[:], w12.ap()[l].rearrange(
                    "w p c t -> p w c t"))
                z2sum = stt.tile([128, FC, NBLK], F32, name=f"z2s{l}",
                                 tag="z2s")
                wsum = stt.tile([128, FC, NAB], F32, name=f"ws{l}", tag="ws")

                # ---- D1A + MASK per group ----
                for g in range(NG):
                    s0 = g * GRP
                    hWg = hw2[:, g % 3]
                    mta = mskp.tile([128, GRP, ROI], F16, name=f"mta{l}_{g}",
                                    tag="mta")
                    mtb = mskp.tile([128, GRP, ROI], F16, name=f"mtb{l}_{g}",
                                    tag="mtb")
                    nc.sync.dma_start(
                        mta[:], mk.ap()[l, s0:s0 + GRP, 0:128, :].rearrange(
                            "s j i -> j s i"))
                    nc.sync.dma_start(
                        mtb[0:72], mk.ap()[l, s0:s0 + GRP, 128:200,
                                           :].rearrange("s j i -> j s i"))
                    for si in range(GRP):
                        c0 = (s0 + si) * ROI
                        for ch in range(2):
                            rows = 128 if ch == 0 else 72
                            ps = dps.tile([128, 512], F32,
                                          name=f"d1{l}_{g}_{si}_{ch}",
                                          tag="dpst")
                            for fi in range(FC):
                                nc.tensor.matmul(
                                    ps[:rows, :],
                                    bufC[:, fi, c0 + ch * 128:
                                         c0 + ch * 128 + rows],
                                    wt[:, 0, fi, :],
                                    start=(fi == 0), stop=(fi == FC - 1))
                            dst = hWg[:rows, si, ch, :]
                            if (si + ch) % 2 == 0:
                                nc.scalar.activation(dst, ps[:rows, :], AF.Copy)
                            else:
                                nc.vector.tensor_scalar(dst, ps[:rows, :],
                                                        1.0, 0.0,
                                                        A.mult, A.add)
                    if DBG and l == 0 and g == 0:
                        nc.sync.dma_start(dbg_hw.ap(), hWg[:])
                    for si in range(GRP):
                        c0 = (s0 + si) * ROI
                        for half in range(2):
                            zps = aps.tile([128, 2, ROI], F32,
                                           name=f"zp{l}_{g}_{si}_{half}",
                                           tag="apst")
                            for sub in range(2):
                                fo = half * 2 + sub
                                nc.tensor.matmul(
                                    zps[:, sub, :],
                                    hWg[:, si, 0, fo * 128:(fo + 1) * 128],
                                    mta[:, si, :], start=True, stop=False)
                                nc.tensor.matmul(
                                    zps[:, sub, :],
                                    hWg[0:72, si, 1, fo * 128:(fo + 1) * 128],
                                    mtb[0:72, si, :], start=False, stop=True,
                                    skip_group_check=True)
                            dst = bufA[:, half * 2:half * 2 + 2, c0:c0 + ROI]
                            if (si + half) % 2 == 0:
                                nc.scalar.activation(dst, zps[:], AF.Copy)
                            else:
                                nc.vector.tensor_scalar(dst, zps[:], 1.0, 0.0,
                                                        A.mult, A.add)

                # ---- BN1 (subset stats) + ELU1 ----
                if DBG and l == 0:
                    nc.sync.dma_start(dbg_h.ap(), bufC[:])
                    nc.sync.dma_start(dbg_z1.ap(), bufA[:])
                s1sum = sum_subset(bufA, SUB1, f"z1_{l}")
                s1sq = sumsq_subset(bufA, SUB1, f"z1_{l}")
                s1, t1, m1 = ar_stats(s1sum[:], s1sq[:], NCORES * SUB1,
                                      l, 0, 1, f"a{l}")
                elu_shift(bufA, bufB, s1, t1, m1, f"a{l}")
                if DBG and l == 0:
                    nc.sync.dma_start(dbg_u.ap(), bufB[:])

                # ---- D2 (copies carry z2 sums) ----
                for rb in range(NBLK):
                    off = rb * 512
                    n = min(512, RPC - off)
                    for fo in range(FC):
                        ps = dps.tile([128, 512], F32, name=f"d2{l}_{rb}_{fo}",
                                      tag="dpst")
                        for fi in range(FC):
                            nc.tensor.matmul(
                                ps[:, :n],
                                wt[:, 1, fi, fo * 128:(fo + 1) * 128],
                                bufB[:, fi, off:off + n],
                                start=(fi == 0), stop=(fi == FC - 1))
                        if (rb + fo) % 2 == 0:
                            nc.scalar.activation(
                                bufA[:, fo, off:off + n], ps[:, :n], AF.Copy,
                                accum_out=z2sum[:, fo, rb:rb + 1])
                        else:
                            nc.vector.tensor_scalar(
                                bufA[:, fo, off:off + n], ps[:, :n], 1.0, 0.0,
                                A.mult, A.add,
                                accum_out=z2sum[:, fo, rb:rb + 1])

                # ---- BN2 + ELU2 (accums carry w sums) ----
                if DBG and l == 0:
                    nc.sync.dma_start(dbg_z2.ap(), bufA[:])
                s2sumr = stt.tile([128, FC], F32, name=f"s2r{l}", tag="s2r")
                nc.vector.tensor_reduce(s2sumr[:], z2sum[:, :, 0:NBLK],
                                        mybir.AxisListType.X, A.add)
                s2sq = sumsq_subset(bufA, SUB2, f"z2_{l}")
                s2, t2, m2 = ar_stats(s2sumr[:], s2sq[:], NCORES * SUB2,
                                      l, 2, 3, f"b{l}")
                elu_shift(bufA, bufB, s2, t2, m2, f"b{l}", acc=wsum)

                # ---- BN3 + ELU3 ----
                if DBG and l == 0:
                    nc.sync.dma_start(dbg_w.ap(), bufB[:])
                s3sumr = stt.tile([128, FC], F32, name=f"s3r{l}", tag="s3r")
                nc.vector.tensor_reduce(s3sumr[:], wsum[:, :, 0:NAB],
                                        mybir.AxisListType.X, A.add)
                s3sq = sumsq_subset(bufB, SUB3, f"w_{l}")
                s3, t3, _ = ar_stats(s3sumr[:], s3sq[:], NCORES * SUB3,
                                     l, 4, 5, f"c{l}")
                elu_exact(bufB, bufC, s3, t3, f"c{l}")
                if DBG and l == 0:
                    nc.sync.dma_start(dbg_h2.ap(), bufC[:])

            # ---- final head ----
            xmT = big.tile([128, FC, S], F32)
            for fcx in range(FC):
                nc.vector.tensor_reduce(
                    xmT[:, fcx, :],
                    bufC[:, fcx, :].rearrange("p (s r) -> p s r", r=ROI),
                    mybir.AxisListType.X, A.add)
            gin = dram.tile([128, FC * S], F32, name="gin")
            gout = dram.tile([NCORES, 128, FC * S], F32, name="gout",
                             addr_space="Shared")
            nc.sync.dma_start(gin[:], xmT[:].rearrange("p c s -> p (c s)"))
            nc.gpsimd.collective_compute(
                "AllGather", A.bypass, ins=[gin[:].opt()], outs=[gout[:].opt()],
                replica_groups=[list(range(NCORES))])
            xa = big.tile([128, FC, NCORES, S], F16)
            nc.gpsimd.dma_start(
                xa[:], gout[:].rearrange("r p (c s) -> p c r s", c=FC))
            zt = big.tile([128, 2, B], F32)
            st6f = stt.tile([128, 2, 1, 6], F32, name="st6f", tag="st6f")
            for fo in range(2):
                ps = aps.tile([128, B], F32, name=f"fps{fo}", tag="apst")
                for fi in range(FC):
                    nc.tensor.matmul(ps[:], wm1t[:, fi, fo * 128:(fo + 1) * 128],
                                     xa[:, fi], start=(fi == 0),
                                     stop=(fi == FC - 1))
                nc.scalar.activation(zt[:, fo, :], ps[:], AF.Copy)
                nc.vector.bn_stats(st6f[:, fo, 0], zt[:, fo, :])
            agf = stt.tile([128, 2, 2], F32, name="agf")
            for fo in range(2):
                nc.vector.bn_aggr(agf[:, fo], st6f[:, fo, 0])
            vgf = stt.tile([128, 2], F32, name="vgf")
            nc.vector.tensor_copy(vgf[:], agf[:, :, 1:2].rearrange(
                "p c o -> p (c o)"))
            nc.vector.tensor_scalar(vgf[:], vgf[:], 1.0, BN_EPS, A.mult, A.add)
            nc.scalar.activation(vgf[:], vgf[:], AF.Ln, bias=0.0, scale=1.0)
            nc.scalar.activation(vgf[:], vgf[:], AF.Exp, bias=0.0, scale=-0.5)
            sf = stt.tile([128, 2], F32, name="sf")
            tf = stt.tile([128, 2], F32, name="tf")
            nc.vector.tensor_tensor(sf[:], vgf[:], fbnt[:, 0:2], A.mult)
            nc.vector.tensor_tensor(tf[:], agf[:, :, 0:1].rearrange(
                "p c o -> p (c o)"), sf[:], A.mult)
            nc.vector.tensor_tensor(tf[:], fbnt[:, 2:4], tf[:], A.subtract)
            rt = big.tile([128, 2, B], F16)
            for fo in range(2):
                nc.scalar.activation(rt[:, fo, :], zt[:, fo, :], AF.Relu,
                                     bias=tf[:, fo:fo + 1],
                                     scale=sf[:, fo:fo + 1])
            psy = aps.tile([128, B], F32, name="psy", tag="apst")
            for fo in range(2):
                nc.tensor.matmul(psy[0:2, :], wm2t[:, fo, :], rt[:, fo, :],
                                 start=(fo == 0), stop=(fo == 1))
            ysb = big.tile([128, B], F32)
            nc.vector.tensor_scalar(ysb[0:2, :], psy[0:2, :], 1.0,
                                    fbnt[0:2, 4:5], A.mult, A.add)
            nc.sync.dma_start(y.ap().rearrange("b t -> t b"), ysb[0:2, :])
    nc.compile()
    return nc


_NC_CACHE = None


def _get_nc():
    global _NC_CACHE
    if _NC_CACHE is None:
        _NC_CACHE = build_nc()
    return _NC_CACHE


def _prep_inputs(x, a, eps, W1, W2, gl_, bl_, g1, be1, g2, be2,
                 gm, betam, Wm1, bm2, Wm2):
    f16 = np.float16
    mask = (np.asarray(a) != 0).astype(np.float32)          # [b, i, j]
    maskT = np.ascontiguousarray(mask.transpose(0, 2, 1))   # [b, j, i]
    eye = np.eye(ROI, dtype=np.float32)
    mkv = np.empty((L, B, ROI, ROI), dtype=f16)
    for l in range(L):
        mkv[l] = (maskT + float(eps[l]) * eye).astype(f16)
    w12 = np.empty((L, 2, 128, FC, T), dtype=f16)
    for l in range(L):
        w12[l, 0] = np.asarray(W1[l]).reshape(FC, 128, T).transpose(1, 0, 2)
        w12[l, 1] = np.asarray(W2[l]).reshape(FC, 128, T).transpose(1, 0, 2)
    bnpv = np.empty((L, 6, 128, FC), dtype=np.float32)
    for l in range(L):
        for k, p in enumerate((g1[l], be1[l], g2[l], be2[l], gl_[l], bl_[l])):
            bnpv[l, k] = np.asarray(p).reshape(FC, 128).T
    wm1p = (np.asarray(Wm1) / ROI).reshape(FC, 128, 256).transpose(
        1, 0, 2).astype(f16)
    wm2p = np.asarray(Wm2).reshape(2, 128, 2).transpose(1, 0, 2).astype(f16)
    fbnv = np.zeros((128, 5), dtype=np.float32)
    fbnv[:, 0:2] = np.asarray(gm).reshape(2, 128).T
    fbnv[:, 2:4] = np.asarray(betam).reshape(2, 128).T
    fbnv[0:2, 4] = np.asarray(bm2)
    xf = np.asarray(x).astype(f16)                           # [b, roi, T]
    return xf, mkv, w12, bnpv, wm1p, wm2p, fbnv


def make_in_maps(inputs):
    xf, mkv, w12, bnpv, wm1p, wm2p, fbnv = _prep_inputs(
        inputs['x'], inputs['a'], inputs['eps'], inputs['W1'], inputs['W2'],
        inputs['gl'], inputs['bl'], inputs['g1'], inputs['be1'], inputs['g2'],
        inputs['be2'], inputs['gm'], inputs['betam'], inputs['Wm1'],
        inputs['bm2'], inputs['Wm2'])
    in_maps = []
    for c in range(NCORES):
        sl = slice(c * S, (c + 1) * S)
        xc = xf[sl]                                          # [S, ROI, T]
        xt = np.ascontiguousarray(
            xc.transpose(2, 0, 1).reshape(FC, 128, RPC).transpose(1, 0, 2))
        in_maps.append({
            "xrt": xt,
            "mk": np.ascontiguousarray(mkv[:, sl]),
            "w12": w12, "bnp": bnpv, "wm1": wm1p, "wm2": wm2p, "fbn": fbnv,
        })
    return in_maps


def kernel(x, a, eps, W1, b1, g1, be1, W2, b2, g2, be2, gl, bl,
           Wm1, bm1, gm, betam, Wm2, bm2):
    in_maps = make_in_maps(dict(x=x, a=a, eps=eps, W1=W1, W2=W2, gl=gl, bl=bl,
                                g1=g1, be1=be1, g2=g2, be2=be2, gm=gm,
                                betam=betam, Wm1=Wm1, bm2=bm2, Wm2=Wm2))
    nc = _get_nc()
    res = run_bass_kernel_spmd(nc, in_maps, core_ids=list(range(NCORES)))
    return res.results[0]["y"].astype(np.float32)
